# revision 23
# baseline (speedup 1.0000x reference)
"""Trainium2 Bass kernel for a 2-layer heterogeneous GraphSAGE
(DrugRepurposing HeteroGNN): disease/gene/drug nodes, 4 edge types,
SAGEConv(mean) per type, HeteroConv sum, ReLU between layers.

Strategy (8-core SPMD, destination-node sharding):
  * Destination nodes of each type are split into 8 equal contiguous
    shards. Source feature tables are replicated (layer 1) or
    all-gathered (layer 2), so no per-edge communication is needed.
  * Per edge type, each destination node's incoming edges are padded to a
    fixed slot count S (64 or 32, dividing 128). A 128-edge tile then
    covers exactly 128/S consecutive destination nodes, so the
    segment-sum is a matmul with a *constant* block-indicator lhsT.
    Gathered rows for padding slots point at an all-zero table row.
  * Edges beyond S per node ("cleanup", a few %) are handled per
    128-node block with a one-hot lhsT built on the vector engine from a
    per-edge local-destination-row array (255 = inactive).
  * Gathers use the GPSIMD dma_gather custom DMA with int16 indices over
    bf16 tables (256 B rows).  All aggregation/matmul accumulation is
    fp32 in PSUM; mean division is fused into the PSUM eviction as a
    per-partition 1/deg scale; bias (+ReLU for layer 1) is fused into
    the dense eviction.
  * The dense SAGE transform is computed feature-major:
    out_T = Wl^T @ mean_T + Wr^T @ x_T + b, with mean_T produced by a
    PE transpose of the node-major aggregation block.
  * Between layers, per-core activations are written node-major as bf16
    and exchanged with 3 AllGather collectives to form layer-2 tables.
Host-side (free) work: edge bucketing/sorting, slot/index-stream
construction, weight/feature transposes, output unpadding.
"""

import os
import sys
from contextlib import ExitStack

import numpy as np

sys.path.insert(0, "/opt/trn_rl_repo")

import ml_dtypes

BF16 = ml_dtypes.bfloat16

P = 128
HID = 128
NCORES = 8

FULL_CFG = dict(
    node_n={"d": 20000, "g": 30000, "r": 15000},
    S={"dg": 64, "gd": 64, "gdr": 64, "drg": 32},
)

# conv name -> (src_type, dst_type)
CONV_TYPES = [("dg", "d", "g"), ("gd", "g", "d"), ("gdr", "g", "r"), ("drg", "r", "g")]
EDGE_KEY = {"dg": "ei_dg", "gd": "ei_gd", "gdr": "ei_gdr", "drg": "ei_drg"}


# ----------------------------------------------------------------------------
# host-side preprocessing
# ----------------------------------------------------------------------------


class Cfg:
    def __init__(self, raw):
        self.node_n = raw["node_n"]
        self.S = raw["S"]
        self.loc_n = {t: n // NCORES for t, n in self.node_n.items()}
        self.nblk = {t: (self.loc_n[t] + P - 1) // P for t in self.node_n}
        self.pad_n = {t: self.nblk[t] * P for t in self.node_n}
        self.convs = [(nm, st, dt, self.S[nm]) for nm, st, dt in CONV_TYPES]


def build_graph_plan(cfg, edge_arrays):
    plan = {}
    for name, st, dt, S in cfg.convs:
        ei = np.asarray(edge_arrays[EDGE_KEY[name]])
        src = ei[0].astype(np.int64)
        dst = ei[1].astype(np.int64)
        n_loc = cfg.loc_n[dt]
        nblk = cfg.nblk[dt]
        pad_n = cfg.pad_n[dt]
        per_core = []
        cl_counts = np.zeros((NCORES, nblk), np.int64)
        deg_all = []
        for c in range(NCORES):
            lo = c * n_loc
            m = (dst >= lo) & (dst < lo + n_loc)
            s_c = src[m]
            d_c = (dst[m] - lo).astype(np.int64)
            order = np.argsort(d_c, kind="stable")
            s_c = s_c[order]
            d_c = d_c[order]
            deg = np.bincount(d_c, minlength=pad_n).astype(np.int64)
            deg_all.append(deg)
            starts = np.zeros(pad_n + 1, np.int64)
            np.cumsum(deg, out=starts[1:])
            slots = np.full((pad_n, S), -1, np.int64)
            take = np.minimum(deg, S)
            node_ids = np.repeat(np.arange(pad_n), take)
            cum = np.cumsum(take) - take
            slot_pos = np.arange(node_ids.shape[0]) - np.repeat(cum, take)
            src_pos = np.repeat(starts[:-1], take) + slot_pos
            slots[node_ids, slot_pos] = s_c[src_pos]
            cleanup = [[] for _ in range(nblk)]
            for nl in np.nonzero(deg > S)[0]:
                b = nl // P
                row = nl % P
                for e in range(starts[nl] + S, starts[nl + 1]):
                    cleanup[b].append((row, int(s_c[e])))
                cl_counts[c, b] += deg[nl] - S
            per_core.append((slots, cleanup))
        t_cl = ((cl_counts + P - 1) // P).max(axis=0).astype(np.int64)
        plan[name] = dict(
            S=S, st=st, dt=dt, nblk=nblk, per_core=per_core, t_cl=t_cl, deg=deg_all
        )
    return plan


def stream_layout(cfg, plan):
    """(conv, block, n_a_tiles, n_cl_tiles, offset) segments; SPMD-uniform."""
    segs = []
    off = 0
    for name, st, dt, S in cfg.convs:
        cv = plan[name]
        for b in range(cv["nblk"]):
            n_a = S
            n_cl = int(cv["t_cl"][b])
            segs.append((name, b, n_a, n_cl, off))
            off += (n_a + n_cl) * P
    return segs, off


def build_index_stream(cfg, plan, segs, tot, core, layer):
    out = np.zeros(tot, np.int64)
    for name, b, n_a, n_cl, off in segs:
        cv = plan[name]
        S = cv["S"]
        st = cv["st"]
        slots, cleanup = cv["per_core"][core]
        if layer == 1:
            zero = cfg.node_n[st]
            tix = lambda s: s
        else:
            n = cfg.loc_n[st]
            zero = n  # core 0's zero row
            tix = lambda s: (s // n) * (n + 1) + (s % n)
        npt = P // S
        blk = slots[b * P : (b + 1) * P]
        i = np.arange(n_a * P)
        node = npt * (i // P) + (i % P) // S
        slot = (i % P) % S
        vals = blk[node, slot]
        out[off : off + n_a * P] = np.where(vals >= 0, tix(np.maximum(vals, 0)), zero)
        cl = cleanup[b]
        cl_vals = np.full(n_cl * P, zero, np.int64)
        for j, (_row, s) in enumerate(cl):
            cl_vals[j] = tix(np.int64(s))
        out[off + n_a * P : off + (n_a + n_cl) * P] = cl_vals
    assert out.max() < 32768
    return out.astype(np.int16)


def build_cl_dloc(cfg, plan, segs, core):
    ncl_total = sum(s[3] for s in segs)
    dloc = np.full((P, max(ncl_total, 1)), 255.0, np.float32)
    k = 0
    for name, b, n_a, n_cl, off in segs:
        cl = plan[name]["per_core"][core][1][b]
        for j, (row, _s) in enumerate(cl):
            dloc[j % P, k + j // P] = float(row)
        k += n_cl
    return dloc, max(ncl_total, 1)


def build_deginv(cfg, plan, core):
    cols = []
    for name, st, dt, S in cfg.convs:
        deg = plan[name]["deg"][core].astype(np.float64)
        inv = np.where(deg > 0, 1.0 / np.maximum(deg, 1), 0.0)
        cols.append(inv.reshape(-1, P).T)
    return np.concatenate(cols, axis=1).astype(np.float32)


def wrap_idx(idx):
    """[TOT] -> [128, TOT/16] int16 (16-row wrap, replicated 8x)."""
    w = np.ascontiguousarray(idx.reshape(-1, 16).T)
    return np.tile(w, (8, 1))


def const_lhst_wide(S):
    """[128, npt*(S-1)+128]; slice [:, W0-npt*ti : +128] is the tile-ti
    block-indicator lhsT mapping row e -> column npt*ti + e//S."""
    npt = P // S
    w0 = npt * (S - 1)
    m = np.zeros((P, w0 + P), np.float32)
    m[np.arange(P), w0 + np.arange(P) // S] = 1.0
    return m


# ----------------------------------------------------------------------------
# device program
# ----------------------------------------------------------------------------


def build_program(cfg, plan, segs, tot, ncl_total, debug_taps=False, layers=(1, 2)):
    import concourse.bacc as bacc
    import concourse.mybir as mybir
    import concourse.tile as tile

    f32 = mybir.dt.float32
    bf16 = mybir.dt.bfloat16
    i16 = mybir.dt.int16
    AF = mybir.ActivationFunctionType
    ALU = mybir.AluOpType

    nc = bacc.Bacc("TRN2", target_bir_lowering=False, debug=False, num_devices=NCORES)

    nb_tot = sum(cfg.nblk[dt] for _, _, dt, _ in cfg.convs)
    xtc = sum(cfg.pad_n[t] for t in "dgr")
    toff = {}
    o = 0
    for t in "dgr":
        toff[t] = o
        o += cfg.pad_n[t]

    # --- DRAM tensors ---
    idx_dram = {
        l: nc.dram_tensor(f"idx{l}", [P, tot // 16], i16, kind="ExternalInput")
        for l in (1, 2)
    }
    dloc_dram = nc.dram_tensor("dloc", [P, ncl_total], f32, kind="ExternalInput")
    deginv_dram = nc.dram_tensor("deginv", [P, nb_tot], f32, kind="ExternalInput")
    xt_dram = nc.dram_tensor("xT", [P, xtc], f32, kind="ExternalInput")
    ident_dram = nc.dram_tensor("ident", [P, P], f32, kind="ExternalInput")
    iota_dram = nc.dram_tensor("iota", [P, P], bf16, kind="ExternalInput")
    lhst_dram = {
        S: nc.dram_tensor(
            f"lhst{S}", [P, (P // S) * (S - 1) + P], bf16, kind="ExternalInput"
        )
        for S in sorted(set(cfg.S.values()))
    }
    tab1 = {
        t: nc.dram_tensor(
            f"tab1_{t}", [cfg.node_n[t] + 1, HID], bf16, kind="ExternalInput"
        )
        for t in "dgr"
    }
    wl_dram = {
        (l, nm): nc.dram_tensor(f"wl_{nm}_{l}", [P, HID], f32, kind="ExternalInput")
        for l in (1, 2)
        for nm, _, _ in CONV_TYPES
    }
    wr_dram = {
        (l, t): nc.dram_tensor(f"wr_{t}_{l}", [P, HID], f32, kind="ExternalInput")
        for l in (1, 2)
        for t in "dgr"
    }
    bias_dram = {
        (l, t): nc.dram_tensor(f"bias_{t}_{l}", [P, 1], f32, kind="ExternalInput")
        for l in (1, 2)
        for t in "dgr"
    }
    stage = {
        t: nc.dram_tensor(
            f"stage_{t}", [cfg.loc_n[t] + 1, HID], bf16, kind="Internal"
        )
        for t in "dgr"
    }
    tab2 = {
        t: nc.dram_tensor(
            f"tab2_{t}",
            [(cfg.loc_n[t] + 1) * NCORES, HID],
            bf16,
            kind="Internal",
            addr_space="Shared",
        )
        for t in "dgr"
    }
    out_dram = {
        t: nc.dram_tensor(
            f"out_{t}", [cfg.nblk[t], P, HID], f32, kind="ExternalOutput"
        )
        for t in "dgr"
    }
    if debug_taps:
        dbg_g = nc.dram_tensor("dbg_g", [P, 8, HID], f32, kind="ExternalOutput")
        dbg_mean = nc.dram_tensor("dbg_mean", [P, HID], f32, kind="ExternalOutput")
        dbg_mt = nc.dram_tensor("dbg_mt", [P, HID], f32, kind="ExternalOutput")
        dbg_xt = nc.dram_tensor("dbg_xt", [P, HID], f32, kind="ExternalOutput")
        dbg_ot = nc.dram_tensor("dbg_ot", [P, HID], f32, kind="ExternalOutput")

    rg = [list(range(NCORES))]
    max_c = max(s[2] + s[3] for s in segs)

    with tile.TileContext(nc) as tc, ExitStack() as ctx:
        const = ctx.enter_context(tc.tile_pool(name="const", bufs=1))
        idxp = ctx.enter_context(tc.tile_pool(name="idxp", bufs=4))
        gath = ctx.enter_context(tc.tile_pool(name="gath", bufs=3))
        ohp = ctx.enter_context(tc.tile_pool(name="ohp", bufs=3))
        meanp = ctx.enter_context(tc.tile_pool(name="meanp", bufs=3))
        mtp_sb = ctx.enter_context(tc.tile_pool(name="mtp_sb", bufs=3))
        stgp = ctx.enter_context(tc.tile_pool(name="stgp", bufs=3))
        outp = ctx.enter_context(tc.tile_pool(name="outp", bufs=3))
        # PSUM pool tiles are bank-granular: 8 banks total.
        aggp = ctx.enter_context(tc.tile_pool(name="aggp", bufs=4, space="PSUM"))
        mtpp = ctx.enter_context(tc.tile_pool(name="mtpp", bufs=2, space="PSUM"))
        dpp = ctx.enter_context(tc.tile_pool(name="dpp", bufs=2, space="PSUM"))
        stpp = mtpp

        def load_const(dram, shape, dtype):
            t = const.tile(shape, dtype, tag=dram.name)
            nc.sync.dma_start(t[:], dram[:])
            return t

        ident_sb = load_const(ident_dram, [P, P], f32)
        iota_sb = load_const(iota_dram, [P, P], bf16)
        lhst_sb = {
            S: load_const(d, [P, (P // S) * (S - 1) + P], bf16)
            for S, d in lhst_dram.items()
        }
        dloc_sb = load_const(dloc_dram, [P, ncl_total], f32)
        deginv_sb = load_const(deginv_dram, [P, nb_tot], f32)
        xt_sb = load_const(xt_dram, [P, xtc], f32)
        wl_sb = {k: load_const(d, [P, HID], f32) for k, d in wl_dram.items()}
        wr_sb = {k: load_const(d, [P, HID], f32) for k, d in wr_dram.items()}
        bias_sb = {k: load_const(d, [P, 1], f32) for k, d in bias_dram.items()}

        out1t_sb = const.tile([P, xtc], f32, tag="out1t")
        meantdg_sb = const.tile([P, cfg.pad_n["g"]], f32, tag="meantdg")

        zrow = const.tile([1, HID], bf16, tag="zrow")
        nc.vector.memset(zrow[:], 0)
        for t in "dgr":
            nc.sync.dma_start(stage[t][cfg.loc_n[t] : cfg.loc_n[t] + 1, :], zrow[:])

        def dense_block(layer, t, b, terms, xt_cur):
            """terms: list of (w_sb, meanT ap). Adds Wr^T x_T and evicts."""
            dp = dpp.tile([P, P], f32, tag="dp")
            for i, (w, mt) in enumerate(terms):
                nc.tensor.matmul(
                    dp[:], lhsT=w[:], rhs=mt, start=(i == 0), stop=False
                )
            cols = slice(toff[t] + b * P, toff[t] + (b + 1) * P)
            nc.tensor.matmul(
                dp[:],
                lhsT=wr_sb[(layer, t)][:],
                rhs=xt_cur[:, cols],
                start=False,
                stop=True,
            )
            if layer == 1:
                nc.scalar.activation(
                    out1t_sb[:, cols], dp[:], AF.Relu, bias=bias_sb[(1, t)][:, 0:1]
                )
                stp = stpp.tile([P, P], f32, tag="mt")
                nc.tensor.transpose(stp[:], out1t_sb[:, cols], ident_sb[:])
                stg = stgp.tile([P, P], bf16, tag="stg")
                nc.scalar.activation(stg[:], stp[:], AF.Copy)
                rows = min(P, cfg.loc_n[t] - b * P)
                nc.sync.dma_start(
                    stage[t][b * P : b * P + rows, :], stg[:rows, :]
                )
            else:
                ot = outp.tile([P, P], f32, tag="ot")
                nc.scalar.activation(
                    ot[:], dp[:], AF.Identity, bias=bias_sb[(2, t)][:, 0:1]
                )
                nc.sync.dma_start(out_dram[t][b], ot[:])
                if debug_taps and t == "d" and b == 0:
                    nc.sync.dma_start(dbg_ot[:], ot[:])
                    nc.sync.dma_start(dbg_xt[:], xt_cur[:, cols])

        for layer in layers:
            xt_cur = xt_sb if layer == 1 else out1t_sb
            if layer == 1:
                tabs = {t: tab1[t] for t in "dgr"}
            else:
                tabs = {t: tab2[t] for t in "dgr"}
            dcol_base = 0
            clcol = 0
            for name, st, dt, S in cfg.convs:
                cv = plan[name]
                npt = P // S
                for name2, b, n_a, n_cl, off in segs:
                    if name2 != name:
                        continue
                    C = n_a + n_cl
                    idxt = idxp.tile([P, C * 8], i16, tag="idx")
                    nc.sync.dma_start(
                        idxt[:], idx_dram[layer][:, off // 16 : off // 16 + C * 8]
                    )
                    g = gath.tile([P, max_c, HID], bf16, tag="g")
                    nc.gpsimd.dma_gather(
                        g[:, :C, :],
                        tabs[st][:],
                        idxt[:],
                        C * P,
                        C * P,
                        HID,
                        # >64 descriptors per 16-partition stream overflows a
                        # single SWDGE packet and kills the device
                        single_packet=(C * P <= 1024),
                    )
                    agg = aggp.tile([P, P], f32, tag="agg")
                    w0 = npt * (S - 1)
                    for ti in range(n_a):
                        nc.tensor.matmul(
                            agg[:],
                            lhsT=lhst_sb[S][:, w0 - npt * ti : w0 - npt * ti + P],
                            rhs=g[:, ti, :],
                            start=(ti == 0),
                            stop=(ti == n_a - 1 and n_cl == 0),
                        )
                    for ti in range(n_cl):
                        oh = ohp.tile([P, P], bf16, tag="oh")
                        nc.vector.tensor_scalar(
                            oh[:],
                            iota_sb[:],
                            dloc_sb[:, clcol + ti : clcol + ti + 1],
                            None,
                            ALU.is_equal,
                        )
                        nc.tensor.matmul(
                            agg[:],
                            lhsT=oh[:],
                            rhs=g[:, n_a + ti, :],
                            start=False,
                            stop=(ti == n_cl - 1),
                        )
                    clcol += n_cl
                    mean = meanp.tile([P, P], f32, tag="mean")
                    nc.scalar.mul(
                        mean[:], agg[:], deginv_sb[:, dcol_base + b : dcol_base + b + 1]
                    )
                    if debug_taps and layer == 2 and name == "gd" and b == 0:
                        gdbg = const.tile([P, 8, HID], f32, tag="gdbg")
                        nc.vector.tensor_copy(gdbg[:], g[:, :8, :])
                        nc.sync.dma_start(dbg_g[:], gdbg[:])
                        nc.sync.dma_start(dbg_mean[:], mean[:])
                    mt_ps = mtpp.tile([P, P], f32, tag="mt")
                    nc.tensor.transpose(mt_ps[:], mean[:], ident_sb[:])
                    if name == "dg":
                        nc.vector.tensor_copy(
                            meantdg_sb[:, b * P : (b + 1) * P], mt_ps[:]
                        )
                    else:
                        mt = mtp_sb.tile([P, P], f32, tag="mtsb")
                        nc.vector.tensor_copy(mt[:], mt_ps[:])
                        if debug_taps and layer == 2 and name == "gd" and b == 0:
                            nc.sync.dma_start(dbg_mt[:], mt[:])
                        if name == "drg":
                            terms = [
                                (wl_sb[(layer, "drg")], mt[:]),
                                (
                                    wl_sb[(layer, "dg")],
                                    meantdg_sb[:, b * P : (b + 1) * P],
                                ),
                            ]
                        else:
                            terms = [(wl_sb[(layer, name)], mt[:])]
                        dense_block(layer, dt, b, terms, xt_cur)
                dcol_base += cv["nblk"]
            if layer == 1 and 2 in layers:
                for t in "dgr":
                    nc.gpsimd.collective_compute(
                        "AllGather",
                        mybir.AluOpType.bypass,
                        replica_groups=rg,
                        ins=[stage[t][:]],
                        outs=[tab2[t][:]],
                    )

    nc.compile()
    return nc


# ----------------------------------------------------------------------------
# in_maps + output assembly
# ----------------------------------------------------------------------------


def make_in_maps(cfg, plan, segs, tot, ncl_total, inputs):
    xs = {
        "d": np.asarray(inputs["x_disease"], np.float32),
        "g": np.asarray(inputs["x_gene"], np.float32),
        "r": np.asarray(inputs["x_drug"], np.float32),
    }
    tabs = {
        t: np.vstack([xs[t], np.zeros((1, HID), np.float32)]).astype(BF16)
        for t in xs
    }
    params = {1: inputs["params1"], 2: inputs["params2"]}
    common = {}
    for t in "dgr":
        common[f"tab1_{t}"] = tabs[t]
    common["ident"] = np.eye(P, dtype=np.float32)
    common["iota"] = np.tile(np.arange(P, dtype=np.float32), (P, 1)).astype(BF16)
    for S in sorted(set(cfg.S.values())):
        common[f"lhst{S}"] = const_lhst_wide(S).astype(BF16)
    for l in (1, 2):
        pr = params[l]
        for nm, _, _ in CONV_TYPES:
            common[f"wl_{nm}_{l}"] = np.asarray(pr[nm][0], np.float32)
        common[f"wr_g_{l}"] = np.asarray(pr["dg"][2], np.float32) + np.asarray(
            pr["drg"][2], np.float32
        )
        common[f"wr_d_{l}"] = np.asarray(pr["gd"][2], np.float32)
        common[f"wr_r_{l}"] = np.asarray(pr["gdr"][2], np.float32)
        common[f"bias_g_{l}"] = (
            np.asarray(pr["dg"][1], np.float32) + np.asarray(pr["drg"][1], np.float32)
        ).reshape(P, 1)
        common[f"bias_d_{l}"] = np.asarray(pr["gd"][1], np.float32).reshape(P, 1)
        common[f"bias_r_{l}"] = np.asarray(pr["gdr"][1], np.float32).reshape(P, 1)

    in_maps = []
    for c in range(NCORES):
        m = dict(common)
        m["idx1"] = wrap_idx(build_index_stream(cfg, plan, segs, tot, c, 1))
        m["idx2"] = wrap_idx(build_index_stream(cfg, plan, segs, tot, c, 2))
        m["dloc"], _ = build_cl_dloc(cfg, plan, segs, c)
        m["deginv"] = build_deginv(cfg, plan, c)
        xt = np.zeros((P, sum(cfg.pad_n[t] for t in "dgr")), np.float32)
        o = 0
        for t in "dgr":
            n = cfg.loc_n[t]
            xt[:, o : o + n] = xs[t][c * n : (c + 1) * n].T
            o += cfg.pad_n[t]
        m["xT"] = xt
        in_maps.append(m)
    return in_maps


def assemble_outputs(cfg, results):
    out = {}
    names = {"d": "disease", "g": "gene", "r": "drug"}
    for t in "dgr":
        n = cfg.loc_n[t]
        parts = [
            results[c][f"out_{t}"].transpose(0, 2, 1).reshape(-1, HID)[:n]
            for c in range(NCORES)
        ]
        out[names[t]] = np.ascontiguousarray(
            np.concatenate(parts, axis=0), dtype=np.float32
        )
    return out


def prepare(cfg_raw, inputs):
    cfg = Cfg(cfg_raw)
    edge_arrays = {EDGE_KEY[nm]: np.asarray(inputs[EDGE_KEY[nm]]) for nm, _, _ in CONV_TYPES}
    plan = build_graph_plan(cfg, edge_arrays)
    segs, tot = stream_layout(cfg, plan)
    _, ncl_total = build_cl_dloc(cfg, plan, segs, 0)
    nc = build_program(cfg, plan, segs, tot, ncl_total)
    in_maps = make_in_maps(cfg, plan, segs, tot, ncl_total, inputs)
    return cfg, nc, in_maps


def run(cfg_raw, inputs):
    from concourse import bass_utils

    cfg, nc, in_maps = prepare(cfg_raw, inputs)
    res = bass_utils.run_bass_kernel_spmd(nc, in_maps, core_ids=list(range(NCORES)))
    return assemble_outputs(cfg, res.results), res


def kernel(**inputs):
    out, _ = run(FULL_CFG, inputs)
    return out
